# revision 3
# baseline (speedup 1.0000x reference)
"""Trainium2 Bass kernel for nn_AutoregressiveNetwork (MADE-style dense MLP).

Data-parallel over the batch: 8 NeuronCores, 2048 rows each. All 64
per-dimension subnetworks (net0 folded in as {W1=0, B1=w0[0]+b0, W2=I,
W3=v0, B3=c0}) run in feature-major layout (x.T on SBUF partitions).

The baseline was PSUM-evacuation-bound (ACT+DVE are the only engines
that can read PSUM on trn2, and DVE runs at 1x for fp32 sources).  This
version attacks that wall:

  L1/L2 (fp16, 64x64 PE tiling): per group of 4 nets, four concurrent
      K=64/M=64 matmuls (tile_position row+col pairs) compute one net
      each.  L1's bias rides a ones-row of xT2 (x col 63 is dead under
      the autoregressive mask).  L1 lands in a [128,1024] 2-bank PSUM
      tile (one FD-1024 evacuation instead of two FD-512).
  L3 (fp16, 128x32 col tiling): all 64 nets' (scale, trans) rows for one
      512-batch window are accumulated into a SINGLE psum bank: pair p
      goes to col tile p%4 at column-slot p//4 (stationary cols 4s..4s+3
      nonzero only), start=False accumulating matmuls.  Zero-weight
      dummy matmuls settle the bank's has_written bits first, making the
      slot accumulation race-free.  L3 evacuation: ONE FD-512 op per
      window (8x less than per-pair evacuation).

Evacuations are split between ScalarE (activation: relu/bias fused) and
VectorE (tensor_scalar) to balance the two PSUM-capable engines; the
split is tunable via H1_ACT_EVERY.
"""
import numpy as np

from concourse import bacc, tile, mybir
from concourse.bass_utils import run_bass_kernel_spmd

DIM = 64
HID = 64
BATCH = 16384
NCORES = 8
BL = BATCH // NCORES          # 2048 batch rows per core
NT = 512                      # batch window per matmul (one fp32 PSUM bank)
T = BL // NT                  # batch windows per core (4)
G = 16                        # groups of 4 nets
F32 = mybir.dt.float32
F16 = mybir.dt.float16

TRACE = False                 # no NTFF hook in this container
_cache = {}

# every Nth group's h1 evacuation goes to ScalarE instead of VectorE
# (load balancing between the two PSUM-reading engines)
H1_ACT_EVERY = 8


def _build(reps=1):
    key = ("nc", reps)
    if key in _cache:
        return _cache[key]
    nc = bacc.Bacc("TRN2", target_bir_lowering=False, debug=False,
                   num_devices=NCORES)

    xT2 = nc.declare_dram_parameter("xT2", [128, BL], F16, isOutput=False)
    lw1 = nc.declare_dram_parameter("lw1", [128, G * 128], F16, isOutput=False)
    lw2 = nc.declare_dram_parameter("lw2", [128, G * 128], F16, isOutput=False)
    lw3 = nc.declare_dram_parameter("lw3", [128, 33 * 32], F16, isOutput=False)
    bb2 = nc.declare_dram_parameter("bb2", [128, 2 * G], F32, isOutput=False)
    bb3 = nc.declare_dram_parameter("bb3", [128, 1], F32, isOutput=False)
    out = nc.declare_dram_parameter("out", [128, BL], F16, isOutput=True)

    Relu = mybir.ActivationFunctionType.Relu
    Ident = mybir.ActivationFunctionType.Identity
    ADD = mybir.AluOpType.add
    MAX = mybir.AluOpType.max

    with tile.TileContext(nc) as tc:
        with (
            tc.tile_pool(name="const", bufs=1) as cpool,
            tc.tile_pool(name="act", bufs=4) as apool,
            tc.tile_pool(name="ps1", bufs=2, space="PSUM") as pspool1,
            tc.tile_pool(name="ps2", bufs=3, space="PSUM") as pspool2,
            tc.tile_pool(name="ps3", bufs=1, space="PSUM") as pspool3,
        ):
            xw = cpool.tile([128, BL], F16)
            w1 = cpool.tile([128, G * 128], F16)
            w2 = cpool.tile([128, G * 128], F16)
            w3 = cpool.tile([128, 33 * 32], F16)
            b2 = cpool.tile([128, 2 * G], F32)
            b3 = cpool.tile([128, 1], F32)
            osb = cpool.tile([128, BL], F16)
            # chunked loads so group-0 compute starts after ~1/8 of the
            # weights have landed
            for sb, dr in ((b2, bb2), (b3, bb3), (w3, lw3)):
                nc.sync.dma_start(sb[:], dr[:])
            for i in range(8):
                nc.sync.dma_start(xw[:, i * 256:(i + 1) * 256],
                                  xT2[:, i * 256:(i + 1) * 256])
                nc.sync.dma_start(w1[:, i * 256:(i + 1) * 256],
                                  lw1[:, i * 256:(i + 1) * 256])
                nc.sync.dma_start(w2[:, i * 256:(i + 1) * 256],
                                  lw2[:, i * 256:(i + 1) * 256])

            for _rep in range(reps):
              for t in range(T):
                xs = xw[:, t * NT:(t + 1) * NT]

                # one bank collects L3 for all 64 nets of this window;
                # zero-weight dummies (start=True) clear has_written so
                # the per-slot accumulating matmuls below are race-free
                P3 = pspool3.tile([128, NT], F32, tag="ps3")
                for j in range(4):
                    nc.tensor.matmul(P3[32 * j:32 * j + 32, :],
                                     w3[:, 1024:1056], xs[:, :],
                                     tile_position=(0, 32 * j),
                                     start=True, stop=False,
                                     skip_group_check=True)

                for g in range(G):
                    c = g * 128

                    # ---- L1: h1 = relu(x @ W1m); bias rides the
                    # ones-rows (63/127) of xT2.  Four concurrent 64x64
                    # tiles -> one [128,1024] 2-bank psum tile.
                    P1 = pspool1.tile([128, 2 * NT], F32, tag="ps1")
                    nc.tensor.matmul(P1[0:64, 0:NT], w1[0:64, c:c + 64],
                                     xs[0:64, :], tile_position=(0, 0))
                    nc.tensor.matmul(P1[64:128, 0:NT], w1[0:64, c + 64:c + 128],
                                     xs[0:64, :], tile_position=(0, 64))
                    nc.tensor.matmul(P1[0:64, NT:2 * NT], w1[64:128, c:c + 64],
                                     xs[64:128, :], tile_position=(64, 0))
                    nc.tensor.matmul(P1[64:128, NT:2 * NT],
                                     w1[64:128, c + 64:c + 128],
                                     xs[64:128, :], tile_position=(64, 64))

                    s12 = apool.tile([128, 2 * NT], F16, tag="s12")
                    if g % H1_ACT_EVERY == H1_ACT_EVERY - 1:
                        nc.scalar.activation(s12[:], P1[:], Relu, bias=0.0)
                    else:
                        nc.vector.tensor_scalar(s12[:], P1[:], 0.0, None, MAX)

                    # ---- L2: h2 = relu(h1 @ W2 + b2), four concurrent
                    # 64x64 tiles -> two 1-bank psum tiles
                    P2a = pspool2.tile([128, NT], F32, tag="ps2")
                    P2b = pspool2.tile([128, NT], F32, tag="ps2")
                    nc.tensor.matmul(P2a[0:64, :], w2[0:64, c:c + 64],
                                     s12[0:64, 0:NT], tile_position=(0, 0))
                    nc.tensor.matmul(P2a[64:128, :], w2[64:128, c:c + 64],
                                     s12[64:128, 0:NT], tile_position=(64, 64))
                    nc.tensor.matmul(P2b[64:128, :], w2[0:64, c + 64:c + 128],
                                     s12[0:64, NT:2 * NT], tile_position=(0, 64))
                    nc.tensor.matmul(P2b[0:64, :], w2[64:128, c + 64:c + 128],
                                     s12[64:128, NT:2 * NT],
                                     tile_position=(64, 0))

                    t12 = apool.tile([128, 2 * NT], F16, tag="t12")
                    nc.scalar.activation(t12[:, 0:NT], P2a[:], Relu,
                                         bias=b2[:, 2 * g:2 * g + 1])
                    nc.scalar.activation(t12[:, NT:2 * NT], P2b[:], Relu,
                                         bias=b2[:, 2 * g + 1:2 * g + 2])

                    # ---- L3: accumulate this group's two net-pairs into
                    # P3 at their column slots
                    for hh in range(2):
                        p = 2 * g + hh
                        j, s = p % 4, p // 4
                        nc.tensor.matmul(
                            P3[32 * j:32 * j + 32, :],
                            w3[:, 32 * p:32 * p + 32],
                            t12[:, hh * NT:(hh + 1) * NT],
                            tile_position=(0, 32 * j),
                            start=False, stop=(s == 7),
                            skip_group_check=True)

                # ---- L3 evacuation: one FD-512 op for all 64 nets
                off = t * NT
                nc.vector.tensor_scalar(osb[:, off:off + NT], P3[:],
                                        b3[:, 0:1], None, ADD)
                if _rep == reps - 1:
                    nc.sync.dma_start(out[:, off:off + NT],
                                      osb[:, off:off + NT])

    nc.compile()
    _cache[key] = nc
    return nc


def _pair_nets(p):
    """Nets (A, B) held by pair p: A on t12 partitions 0-63, B on 64-127."""
    g, hh = p // 2, p % 2
    return (4 * g, 4 * g + 1) if hh == 0 else (4 * g + 3, 4 * g + 2)


def _pack_weights(w0, b0, v0, c0, W1, B1, W2, B2, W3, B3):
    f = np.float32
    # 64 nets in device order; net 0 is the constant network.
    W1n = np.zeros((64, DIM, HID), f)
    B1n = np.zeros((64, HID), f)
    W2n = np.zeros((64, HID, HID), f)
    B2n = np.zeros((64, HID), f)
    W3n = np.zeros((64, HID, 2), f)
    B3n = np.zeros((64, 2), f)

    mask = (np.arange(DIM)[None, :] < np.arange(1, DIM)[:, None]).astype(f)
    W1n[1:] = W1 * mask[:, :, None]
    B1n[1:] = B1
    W2n[1:] = W2
    B2n[1:] = B2
    W3n[1:] = W3
    B3n[1:] = B3
    # net 0: Linear(1,H)->ReLU->Linear(H,2) with constant ones input
    B1n[0] = w0[0] + b0
    W2n[0] = np.eye(HID, dtype=f)
    W3n[0] = v0
    B3n[0] = c0

    lw1 = np.zeros((128, G * 128), np.float16)
    lw2 = np.zeros((128, G * 128), np.float16)
    bb2 = np.zeros((128, 2 * G), f)
    for g in range(G):
        n = 4 * g
        c = g * 128
        # L1 stationaries, one 64x64 block per net; W1 row 63 is zero for
        # every net (autoregressive mask), so it carries the L1 bias
        # against the ones-row of xT2
        lw1[0:64, c:c + 64] = W1n[n]
        lw1[0:64, c + 64:c + 128] = W1n[n + 1]
        lw1[64:128, c:c + 64] = W1n[n + 2]
        lw1[64:128, c + 64:c + 128] = W1n[n + 3]
        lw1[63, c:c + 64] = B1n[n]
        lw1[63, c + 64:c + 128] = B1n[n + 1]
        lw1[127, c:c + 64] = B1n[n + 2]
        lw1[127, c + 64:c + 128] = B1n[n + 3]
        # L2 stationaries: T0=(0,0)->net n, T10=(64,64)->n+1,
        # T2=(0,64)->n+2 (out parts 64-127), T8=(64,0)->n+3 (out 0-63)
        lw2[0:64, c:c + 64] = W2n[n]
        lw2[64:128, c:c + 64] = W2n[n + 1]
        lw2[0:64, c + 64:c + 128] = W2n[n + 2]
        lw2[64:128, c + 64:c + 128] = W2n[n + 3]
        bb2[:, 2 * g] = np.concatenate([B2n[n], B2n[n + 1]])
        bb2[:, 2 * g + 1] = np.concatenate([B2n[n + 3], B2n[n + 2]])

    # L3: one [128,32] stationary per pair, nonzero only at column slot
    # p//4; block 32 (cols 1024:1056) stays zero for the dummy matmuls
    lw3 = np.zeros((128, 33 * 32), np.float16)
    bb3 = np.zeros((128, 1), f)
    for p in range(32):
        A, B = _pair_nets(p)
        j, s = p % 4, p // 4
        blk = 32 * p
        lw3[0:64, blk + 4 * s:blk + 4 * s + 2] = W3n[A]
        lw3[64:128, blk + 4 * s + 2:blk + 4 * s + 4] = W3n[B]
        base = 32 * j + 4 * s
        bb3[base + 0, 0] = B3n[A, 0]
        bb3[base + 1, 0] = B3n[A, 1]
        bb3[base + 2, 0] = B3n[B, 0]
        bb3[base + 3, 0] = B3n[B, 1]
    return dict(lw1=lw1, lw2=lw2, lw3=lw3, bb2=bb2, bb3=bb3)


def _make_xt2(x_shard):
    xT = np.ascontiguousarray(x_shard.T)          # [64, BL]
    xT2 = np.concatenate([xT, xT], axis=0)        # [128, BL]
    xT2[63, :] = 1.0                              # ones-rows carry L1 bias
    xT2[127, :] = 1.0
    return xT2.astype(np.float16)


def _unpack_out(oc, scales, trans, r0):
    """oc: [128, BL] fp16 device output for one core -> rows r0:r0+BL."""
    ocf = oc.astype(np.float32)
    for p in range(32):
        A, B = _pair_nets(p)
        base = 32 * (p % 4) + 4 * (p // 4)
        scales[r0:r0 + BL, A] = ocf[base + 0]
        trans[r0:r0 + BL, A] = ocf[base + 1]
        scales[r0:r0 + BL, B] = ocf[base + 2]
        trans[r0:r0 + BL, B] = ocf[base + 3]


def kernel(x, w0, b0, v0, c0, W1, B1, W2, B2, W3, B3):
    x = np.asarray(x, np.float32)
    args = [np.asarray(a, np.float32) for a in (w0, b0, v0, c0, W1, B1, W2, B2,
                                                W3, B3)]
    wdict = _pack_weights(*args)

    nc = _build()
    in_maps = []
    for core in range(NCORES):
        xT2 = _make_xt2(x[core * BL:(core + 1) * BL])
        in_maps.append({"xT2": xT2, **wdict})

    res = run_bass_kernel_spmd(nc, in_maps, core_ids=list(range(NCORES)),
                               trace=TRACE)
    kernel.last_exec_time_ns = res.exec_time_ns

    scales = np.empty((BATCH, DIM), np.float32)
    trans = np.empty((BATCH, DIM), np.float32)
    for core in range(NCORES):
        _unpack_out(res.results[core]["out"], scales, trans, core * BL)

    np.clip(scales, -5.0, 5.0, out=scales)
    return scales, trans


# revision 6
# speedup vs baseline: 1.6198x; 1.6198x over previous
"""Trainium2 Bass kernel for nn_AutoregressiveNetwork (MADE-style dense MLP).

Data-parallel over the batch: 8 NeuronCores, 2048 rows each. All 64
per-dimension subnetworks (net0 folded in as {W1=0, B1=w0[0]+b0, W2=I,
W3=v0, B3=c0}) run in feature-major layout (x.T on SBUF partitions).

Micro-benchmarked laws for this stack (bench_micro.py):
  - 4 back-to-back matmuls on disjoint 64x64 PE tiles run ~4-way
    concurrent (59 ns/MM vs 189 serial), IF nothing (mode switches,
    dependency stalls) breaks the chain.
  - PSUM evacuation (the other wall): only ScalarE (~455ns/FD512) and
    VectorE (~560ns/FD512) can read PSUM; fp32 source caps DVE at 1x.

Structure per 512-batch window:
  L1/L2 (fp16, 64x64 tiling): per group of 4 nets, four concurrent
      K=64/M=64 matmuls, one net each, in the micro-proven tile order
      (0,0),(0,64),(64,0),(64,64).  L1 bias rides the ones-rows of xT2
      (x col 63 is dead under the autoregressive mask).
  L3 (fp16, 128x32 col tiling) batched at window end: all 64 nets'
      (scale, trans) rows accumulate into a SINGLE psum bank; pair p ->
      col tile p%4, column-slot p//4 (stationary cols 4s..4s+3 nonzero),
      start=False accumulating matmuls issued round-robin over the four
      col tiles (4-way concurrent).  Zero-weight dummy matmuls settle
      the bank's has_written bits first, making this race-free.  One
      FD-512 evacuation per window covers all 64 nets.
  Only 2 PE mode switches per window (groups <-> col-tiled L3).

Evacuations are split ScalarE/VectorE ~36/29 per window to balance the
measured rates; relu+bias fuse into the evacuation on both engines.
"""
import numpy as np

from concourse import bacc, tile, mybir
from concourse.bass_utils import run_bass_kernel_spmd

DIM = 64
HID = 64
BATCH = 16384
NCORES = 8
BL = BATCH // NCORES          # 2048 batch rows per core
NT = 512                      # batch window per matmul (one fp32 PSUM bank)
T = BL // NT                  # batch windows per core (4)
G = 16                        # groups of 4 nets
F32 = mybir.dt.float32
F16 = mybir.dt.float16

TRACE = False                 # no NTFF hook in this container
_cache = {}


def _build(reps=1):
    key = ("nc", reps)
    if key in _cache:
        return _cache[key]
    nc = bacc.Bacc("TRN2", target_bir_lowering=False, debug=False,
                   num_devices=NCORES)

    xT2 = nc.declare_dram_parameter("xT2", [128, BL], F16, isOutput=False)
    lw1 = nc.declare_dram_parameter("lw1", [128, G * 128], F16, isOutput=False)
    lw2 = nc.declare_dram_parameter("lw2", [128, G * 128], F16, isOutput=False)
    lw3 = nc.declare_dram_parameter("lw3", [128, 33 * 32], F16, isOutput=False)
    bb2 = nc.declare_dram_parameter("bb2", [128, 2 * G], F32, isOutput=False)
    bb3 = nc.declare_dram_parameter("bb3", [128, 1], F32, isOutput=False)
    out = nc.declare_dram_parameter("out", [128, BL], F16, isOutput=True)

    Relu = mybir.ActivationFunctionType.Relu
    ADD = mybir.AluOpType.add
    MAX = mybir.AluOpType.max

    with tile.TileContext(nc) as tc:
        with (
            tc.tile_pool(name="const", bufs=1) as cpool,
            tc.tile_pool(name="act", bufs=3) as apool,
            tc.tile_pool(name="hact", bufs=2) as hpool,
            tc.tile_pool(name="ps1", bufs=4, space="PSUM") as pspool1,
            tc.tile_pool(name="ps2", bufs=3, space="PSUM") as pspool2,
            tc.tile_pool(name="ps3", bufs=1, space="PSUM") as pspool3,
        ):
            xw = cpool.tile([128, BL], F16)
            w1 = cpool.tile([128, G * 128], F16)
            w2 = cpool.tile([128, G * 128], F16)
            w3 = cpool.tile([128, 33 * 32], F16)
            b2 = cpool.tile([128, 2 * G], F32)
            b3 = cpool.tile([128, 1], F32)
            osb = cpool.tile([128, BL], F16)
            # chunked loads so group-0 compute starts after ~1/8 of the
            # weights have landed
            for sb, dr in ((b2, bb2), (b3, bb3), (w3, lw3)):
                nc.sync.dma_start(sb[:], dr[:])
            for i in range(8):
                nc.sync.dma_start(xw[:, i * 256:(i + 1) * 256],
                                  xT2[:, i * 256:(i + 1) * 256])
                nc.sync.dma_start(w1[:, i * 256:(i + 1) * 256],
                                  lw1[:, i * 256:(i + 1) * 256])
                nc.sync.dma_start(w2[:, i * 256:(i + 1) * 256],
                                  lw2[:, i * 256:(i + 1) * 256])

            for _rep in range(reps):
              for t in range(T):
                xs = xw[:, t * NT:(t + 1) * NT]

                # one bank collects L3 for all 64 nets of this window;
                # zero-weight dummies (start=True) settle has_written so
                # the per-slot accumulating matmuls below are race-free
                P3 = pspool3.tile([128, NT], F32, tag="ps3")
                for j in range(4):
                    nc.tensor.matmul(P3[32 * j:32 * j + 32, :],
                                     w3[:, 1024:1056], xs[:, :],
                                     tile_position=(0, 32 * j),
                                     start=True, stop=False,
                                     skip_group_check=True)

                t12s = []
                for g in range(G):
                    c = g * 128

                    # ---- L1: h1 = relu(x @ W1m); bias rides the
                    # ones-rows (63/127) of xT2.  Four concurrent 64x64
                    # tiles in the micro-proven order.
                    Pa = pspool1.tile([128, NT], F32, tag="ps1")
                    Pb = pspool1.tile([128, NT], F32, tag="ps1")
                    nc.tensor.matmul(Pa[0:64, :], w1[0:64, c:c + 64],
                                     xs[0:64, :], tile_position=(0, 0))
                    nc.tensor.matmul(Pa[64:128, :], w1[0:64, c + 64:c + 128],
                                     xs[0:64, :], tile_position=(0, 64))
                    nc.tensor.matmul(Pb[0:64, :], w1[64:128, c:c + 64],
                                     xs[64:128, :], tile_position=(64, 0))
                    nc.tensor.matmul(Pb[64:128, :], w1[64:128, c + 64:c + 128],
                                     xs[64:128, :], tile_position=(64, 64))

                    # h1 evacuation: Pa on DVE, Pb on ACT concurrently
                    s12 = apool.tile([128, 2 * NT], F16, tag="s12")
                    nc.vector.tensor_scalar(s12[:, 0:NT], Pa[:], 0.0, None,
                                            MAX)
                    nc.scalar.activation(s12[:, NT:2 * NT], Pb[:], Relu,
                                         bias=0.0)

                    # ---- L2: h2 = relu(h1 @ W2 + b2), four concurrent
                    # 64x64 tiles; P2a = nets (n, n+1), P2b = (n+3, n+2)
                    P2a = pspool2.tile([128, NT], F32, tag="ps2")
                    P2b = pspool2.tile([128, NT], F32, tag="ps2")
                    nc.tensor.matmul(P2a[0:64, :], w2[0:64, c:c + 64],
                                     s12[0:64, 0:NT], tile_position=(0, 0))
                    nc.tensor.matmul(P2b[64:128, :], w2[0:64, c + 64:c + 128],
                                     s12[0:64, NT:2 * NT],
                                     tile_position=(0, 64))
                    nc.tensor.matmul(P2b[0:64, :], w2[64:128, c + 64:c + 128],
                                     s12[64:128, NT:2 * NT],
                                     tile_position=(64, 0))
                    nc.tensor.matmul(P2a[64:128, :], w2[64:128, c:c + 64],
                                     s12[64:128, 0:NT],
                                     tile_position=(64, 64))

                    # h2 evacuation: ta on DVE (except every 4th group,
                    # rebalancing to ACT 36 / DVE 29 units per window),
                    # tb on ACT
                    ta = hpool.tile([128, NT], F16, tag=f"t12_{2 * g}")
                    tb = hpool.tile([128, NT], F16, tag=f"t12_{2 * g + 1}")
                    if g % 4 == 3:
                        nc.scalar.activation(ta[:], P2a[:], Relu,
                                             bias=b2[:, 2 * g:2 * g + 1])
                    else:
                        nc.vector.tensor_scalar(ta[:], P2a[:],
                                                b2[:, 2 * g:2 * g + 1],
                                                0.0, ADD, MAX)
                    nc.scalar.activation(tb[:], P2b[:], Relu,
                                         bias=b2[:, 2 * g + 1:2 * g + 2])
                    t12s.append(ta)
                    t12s.append(tb)

                # ---- L3 batch: round-robin over the 4 col tiles ->
                # 4-way concurrent, accumulating into P3's column slots
                for p in range(32):
                    j, s = p % 4, p // 4
                    nc.tensor.matmul(P3[32 * j:32 * j + 32, :],
                                     w3[:, 32 * p:32 * p + 32],
                                     t12s[p][:],
                                     tile_position=(0, 32 * j),
                                     start=False, stop=(s == 7),
                                     skip_group_check=True)

                # ---- L3 evacuation: one FD-512 op for all 64 nets
                off = t * NT
                nc.vector.tensor_scalar(osb[:, off:off + NT], P3[:],
                                        b3[:, 0:1], None, ADD)
                if _rep == reps - 1:
                    nc.sync.dma_start(out[:, off:off + NT],
                                      osb[:, off:off + NT])

    nc.compile()
    _cache[key] = nc
    return nc


def _pair_nets(p):
    """Nets (A, B) held by pair p: A on t12 partitions 0-63, B on 64-127."""
    g, hh = p // 2, p % 2
    return (4 * g, 4 * g + 1) if hh == 0 else (4 * g + 3, 4 * g + 2)


def _pack_weights(w0, b0, v0, c0, W1, B1, W2, B2, W3, B3):
    f = np.float32
    # 64 nets in device order; net 0 is the constant network.
    W1n = np.zeros((64, DIM, HID), f)
    B1n = np.zeros((64, HID), f)
    W2n = np.zeros((64, HID, HID), f)
    B2n = np.zeros((64, HID), f)
    W3n = np.zeros((64, HID, 2), f)
    B3n = np.zeros((64, 2), f)

    mask = (np.arange(DIM)[None, :] < np.arange(1, DIM)[:, None]).astype(f)
    W1n[1:] = W1 * mask[:, :, None]
    B1n[1:] = B1
    W2n[1:] = W2
    B2n[1:] = B2
    W3n[1:] = W3
    B3n[1:] = B3
    # net 0: Linear(1,H)->ReLU->Linear(H,2) with constant ones input
    B1n[0] = w0[0] + b0
    W2n[0] = np.eye(HID, dtype=f)
    W3n[0] = v0
    B3n[0] = c0

    lw1 = np.zeros((128, G * 128), np.float16)
    lw2 = np.zeros((128, G * 128), np.float16)
    bb2 = np.zeros((128, 2 * G), f)
    for g in range(G):
        n = 4 * g
        c = g * 128
        # L1 stationaries, one 64x64 block per net; W1 row 63 is zero for
        # every net (autoregressive mask), so it carries the L1 bias
        # against the ones-row of xT2
        lw1[0:64, c:c + 64] = W1n[n]
        lw1[0:64, c + 64:c + 128] = W1n[n + 1]
        lw1[64:128, c:c + 64] = W1n[n + 2]
        lw1[64:128, c + 64:c + 128] = W1n[n + 3]
        lw1[63, c:c + 64] = B1n[n]
        lw1[63, c + 64:c + 128] = B1n[n + 1]
        lw1[127, c:c + 64] = B1n[n + 2]
        lw1[127, c + 64:c + 128] = B1n[n + 3]
        # L2 stationaries: (0,0)->net n, (64,64)->n+1, (0,64)->n+2 (out
        # parts 64-127 of P2b), (64,0)->n+3 (out parts 0-63 of P2b)
        lw2[0:64, c:c + 64] = W2n[n]
        lw2[64:128, c:c + 64] = W2n[n + 1]
        lw2[0:64, c + 64:c + 128] = W2n[n + 2]
        lw2[64:128, c + 64:c + 128] = W2n[n + 3]
        bb2[:, 2 * g] = np.concatenate([B2n[n], B2n[n + 1]])
        bb2[:, 2 * g + 1] = np.concatenate([B2n[n + 3], B2n[n + 2]])

    # L3: one [128,32] stationary per pair, nonzero only at column slot
    # p//4; block 32 (cols 1024:1056) stays zero for the dummy matmuls
    lw3 = np.zeros((128, 33 * 32), np.float16)
    bb3 = np.zeros((128, 1), f)
    for p in range(32):
        A, B = _pair_nets(p)
        j, s = p % 4, p // 4
        blk = 32 * p
        lw3[0:64, blk + 4 * s:blk + 4 * s + 2] = W3n[A]
        lw3[64:128, blk + 4 * s + 2:blk + 4 * s + 4] = W3n[B]
        base = 32 * j + 4 * s
        bb3[base + 0, 0] = B3n[A, 0]
        bb3[base + 1, 0] = B3n[A, 1]
        bb3[base + 2, 0] = B3n[B, 0]
        bb3[base + 3, 0] = B3n[B, 1]
    return dict(lw1=lw1, lw2=lw2, lw3=lw3, bb2=bb2, bb3=bb3)


def _make_xt2(x_shard):
    xT = np.ascontiguousarray(x_shard.T)          # [64, BL]
    xT2 = np.concatenate([xT, xT], axis=0)        # [128, BL]
    xT2[63, :] = 1.0                              # ones-rows carry L1 bias
    xT2[127, :] = 1.0
    return xT2.astype(np.float16)


def _unpack_out(oc, scales, trans, r0):
    """oc: [128, BL] fp16 device output for one core -> rows r0:r0+BL."""
    ocf = oc.astype(np.float32)
    for p in range(32):
        A, B = _pair_nets(p)
        base = 32 * (p % 4) + 4 * (p // 4)
        scales[r0:r0 + BL, A] = ocf[base + 0]
        trans[r0:r0 + BL, A] = ocf[base + 1]
        scales[r0:r0 + BL, B] = ocf[base + 2]
        trans[r0:r0 + BL, B] = ocf[base + 3]


def kernel(x, w0, b0, v0, c0, W1, B1, W2, B2, W3, B3):
    x = np.asarray(x, np.float32)
    args = [np.asarray(a, np.float32) for a in (w0, b0, v0, c0, W1, B1, W2, B2,
                                                W3, B3)]
    wdict = _pack_weights(*args)

    nc = _build()
    in_maps = []
    for core in range(NCORES):
        xT2 = _make_xt2(x[core * BL:(core + 1) * BL])
        in_maps.append({"xT2": xT2, **wdict})

    res = run_bass_kernel_spmd(nc, in_maps, core_ids=list(range(NCORES)),
                               trace=TRACE)
    kernel.last_exec_time_ns = res.exec_time_ns

    scales = np.empty((BATCH, DIM), np.float32)
    trans = np.empty((BATCH, DIM), np.float32)
    for core in range(NCORES):
        _unpack_out(res.results[core]["out"], scales, trans, core * BL)

    np.clip(scales, -5.0, 5.0, out=scales)
    return scales, trans
